# revision 52
# baseline (speedup 1.0000x reference)
"""Multi-head attention Trainium2 kernel (B=8,S=1024,D=1024,H=16,DK=64).

Data-parallel over batch: one batch element per NeuronCore (8 cores).
All matmuls in bf16 (1 PE cycle/row vs 4 for fp32); fp32 PSUM accumulation.
Fully SBUF-resident: no DRAM scratch round trips.

Per core:
  1. x (q/k/v) loaded with casting gpsimd DMAs (fp32->bf16), transposed to
     x^T via XBAR dma_start_transpose, then QP^T/KP^T/VP^T computed directly
     (weights stationary): qpT[p, j, s] = QP^T[j*128+p, s], in two s-strips
     of 512 so heads 0-7 can start after strip 0.
  2. torch-.view head split: head h reads only s in [h*64,(h+1)*64).
     Q_h^T[u, t] = QP^T[b*64+u, h*64+a] for t = a*16+b.  Per head, the
     u-contiguous [64, 1024] operand qT2 is built with 2 strided SBUF
     copies: even b from qpT[0:64], odd b from qpT2[0:64], where qpT2 is a
     partition-swapped twin (qpT2[0:64] = qpT[64:128]) made with one
     SBUF->SBUF DMA per strip.
  3. S^T[k, q] = K_h Q_h^T / 8 via 64-contraction matmuls (k on out
     partitions), exp on ACT into packed causal pt strips (bf16), causal
     diagonal via in-place affine_select.
  4. PV with V augmented by a ones column -> unnormalized out + sumexp in
     PSUM; normalized via DVE reciprocal + Pool multiply (walrus rejects
     divide and any Pool access to PSUM).  Head h's PV/normalize/scatter
     is woven between head h+1's score chunks (1-head pipeline skew).
  5. Per head: transpose hs -> out_h^T, reorder-drain to (b, a) layout,
     two scatter DMAs into OP^T; final projection OP^T x w_o from SBUF.
"""

import numpy as np

import concourse.bass as bass
import concourse.mybir as mybir
import concourse.tile as tile
from concourse import bacc
from concourse.bass_utils import run_bass_kernel_spmd
from concourse.masks import make_identity

B, S, D, H, DK = 8, 1024, 1024, 16, 64
P = 128
F32 = mybir.dt.float32
BF16 = mybir.dt.bfloat16

INTERLEAVE = False
SKEW = 1

# Fallback flags (flip if a feature fails to compile/execute)
USE_XBAR = True        # dma_start_transpose for x^T (else PE transposes)
USE_DIVIDE = True      # tensor_scalar divide (else reciprocal + mul)
USE_CAST_DMA = True    # gpsimd casting DMA loads (else hwdge + convert)

# packed causal pt strip offsets: strip j holds q in [j*128, 1024)
PTOFF = [j * 1024 - 64 * j * (j - 1) for j in range(9)]  # PTOFF[8] = 4608


def _build_nc(mm_mode: str = "bf16"):
    nc = bacc.Bacc(
        "TRN2",
        target_bir_lowering=False,
        debug=False,
        enable_asserts=False,
        num_devices=B,
    )

    q_d = nc.dram_tensor("q", [S, D], F32, kind="ExternalInput")
    k_d = nc.dram_tensor("k", [S, D], F32, kind="ExternalInput")
    v_d = nc.dram_tensor("v", [S, D], F32, kind="ExternalInput")
    wq_d = nc.dram_tensor("w_q", [D, D], F32, kind="ExternalInput")
    wk_d = nc.dram_tensor("w_k", [D, D], F32, kind="ExternalInput")
    wv_d = nc.dram_tensor("w_v", [D, D], F32, kind="ExternalInput")
    wo_d = nc.dram_tensor("w_o", [D, D], F32, kind="ExternalInput")
    out_d = nc.dram_tensor("out", [S, D], F32, kind="ExternalOutput")

    # enlarge the SWDGE descriptor ring: 512-desc casting loads otherwise
    # fill the default 1024-desc carveout and serialize behind transfers
    nc.dynamic_dma_scratch_size = 16384 * 8

    x_ds = {"q": q_d, "k": k_d, "v": v_d}
    w_ds = {"q": wq_d, "k": wk_d, "v": wv_d}

    with tile.TileContext(nc) as tc:
        with (
            tc.tile_pool(name="consts", bufs=1) as consts,
            tc.tile_pool(name="wp", bufs=2) as wp,
            tc.tile_pool(name="xbfp", bufs=2) as xbfp,
            tc.tile_pool(name="xtp", bufs=2) as xtp,
            tc.tile_pool(name="projp", bufs=1) as projp,
            tc.tile_pool(name="proj2p", bufs=1) as proj2p,
            tc.tile_pool(name="optp", bufs=1) as optp,
            tc.tile_pool(name="t2p", bufs=2) as t2p,
            tc.tile_pool(name="ptp", bufs=2) as ptp,
            tc.tile_pool(name="vop", bufs=2) as vop,
            tc.tile_pool(name="hsp", bufs=2) as hsp,
            tc.tile_pool(name="ohtp", bufs=1) as ohtp,
            tc.tile_pool(name="outp", bufs=1) as outp,
            tc.tile_pool(name="psmm", bufs=3, space="PSUM") as psmm,
            tc.tile_pool(name="pspv", bufs=3, space="PSUM") as pspv,
            tc.tile_pool(name="psoh", bufs=2, space="PSUM") as psoh,
        ):
            ident = consts.tile([P, P], BF16, tag="ident")
            make_identity(nc, ident[:])

            # persistent vT2 [80, 1024]: rows 0:64 rewritten per head (V_h^T),
            # rows 64:80 stay 1.0 so the vo XBAR transpose lands the sumexp
            # ones column at position 64 of each 80-wide row.
            vT2 = consts.tile([80, 1024], BF16, tag="vT2")
            nc.gpsimd.memset(vT2[64:80, :], 1.0)

            # persistent projection outputs (transposed) + swapped twins
            pT = {t: projp.tile([P, 8, 1024], BF16, name=f"{t}pT", tag=f"{t}pT") for t in "qkv"}
            pT2 = {t: proj2p.tile([64, 8, 1024], BF16, name=f"{t}pT2", tag=f"{t}pT2") for t in "qkv"}
            opT = optp.tile([P, 8, 1024], BF16, tag="opT")

            drain_flip = [0]

            def drain(out_ap, in_ap, act_ok=False):
                # PSUM->SBUF drains (Pool has no PSUM access path).  ACT
                # copies are only used while ACT is otherwise idle (strip 0):
                # during phase 2 a copy whose matmul is late would
                # head-of-line-block the exp stream on ACT's in-order queue.
                drain_flip[0] += 1
                if act_ok and drain_flip[0] % 2 == 1:
                    nc.scalar.copy(out=out_ap, in_=in_ap)
                else:
                    nc.vector.tensor_copy(out=out_ap, in_=in_ap)

            def load_w(w_d, halves=2):
                # casting DMAs per weight (SWDGE desc-gen on Pool)
                w_sb = wp.tile([P, 8, 1024], BF16, tag="w")
                wsrc = w_d.ap().rearrange("(j p) c -> p j c", p=P)
                csz = 1024 // halves
                for chalf in range(halves):
                    c0 = chalf * csz
                    nc.gpsimd.dma_start(
                        w_sb[:, :, c0 : c0 + csz], wsrc[:, :, c0 : c0 + csz]
                    )
                return w_sb

            def strip_loads(t, st):
                """Load x strip as bf16 and XBAR-transpose into an xT tile."""
                s0 = st * 512
                x_d = x_ds[t]
                xT = xtp.tile([P, 8, 512], BF16, tag="xT")
                for half in range(2):
                    r0 = s0 + half * 256
                    xbf = xbfp.tile([P, 2, 1024], BF16, tag="xbf")
                    nc.gpsimd.dma_start(
                        xbf[:],
                        x_d.ap()[r0 : r0 + 256, :].rearrange(
                            "(s2 p) c -> p s2 c", p=P
                        ),
                    )
                    for s2 in range(2):
                        # one batched XBAR per x-tile: out[p, j, f] =
                        # x^T[j*128+p, f] (verified on HW)
                        s_lo = half * 256 + s2 * P
                        eng = nc.sync if (half * 2 + s2) % 2 == 0 else nc.scalar
                        eng.dma_start_transpose(
                            xT[:, :, s_lo : s_lo + P], xbf[:, s2, :]
                        )
                return xT

            def proj_unit(t, st, w_sb, xT, j):
                """One projection j-group (8 matmuls + drain); the last unit
                also emits the partition-swap twin DMA."""
                s0 = st * 512
                ps = psmm.tile([P, 512], F32, name="mm", tag="mm")
                for dit in range(8):
                    nc.tensor.matmul(
                        ps[:],
                        w_sb[:, dit, j * P : (j + 1) * P],
                        xT[:, dit, :],
                        start=(dit == 0),
                        stop=(dit == 7),
                    )
                drain(pT[t][:, j, s0 : s0 + 512], ps[:], act_ok=(st == 0))
                if j == 7:
                    # partition-swap twin: pT2[0:64] = pT[64:128]
                    eng = nc.sync if st == 0 else nc.scalar
                    eng.dma_start(
                        pT2[t][0:64, :, s0 : s0 + 512],
                        pT[t][64:P, :, s0 : s0 + 512],
                    )

            def strip_proj(t, st, w_sb, xT):
                for j in range(8):
                    proj_unit(t, st, w_sb, xT, j)

            def head_front_a(h):
                """qT2/kT2/vT2 copies and the vo XBAR for head h."""
                t2 = {}
                for idx, t in enumerate("qkv"):
                    if t == "v":
                        dstT = vT2
                    else:
                        dstT = t2p.tile(
                            [64, 1024], BF16, name=f"{t}T2", tag=f"{t}T2"
                        )
                    t2[t] = dstT
                    dview = dstT[0:64, :].rearrange(
                        "u (a b1 b0) -> b0 u b1 a", a=64, b1=8, b0=2
                    )
                    for b0 in range(2):
                        src = (pT[t] if b0 == 0 else pT2[t])[
                            0:64, :, h * 64 : (h + 1) * 64
                        ]
                        eng = nc.gpsimd if (idx * 2 + b0) % 2 == 0 else nc.vector
                        eng.tensor_copy(out=dview[b0], in_=src)

                # vo[p, j, u] = V_h[j*128+p, u] for u<64, 1.0 at u=64 (sumexp)
                vo = vop.tile([P, 8, 80], BF16, name="vo", tag="vo")
                eng = nc.sync if h % 2 == 0 else nc.scalar
                eng.dma_start_transpose(vo[:], vT2[:])
                pt = ptp.tile([P, PTOFF[8]], BF16, name="pt", tag="pt")
                return pt, vo, t2

            def score_chunks(h, pt, t2):
                """Emitter thunks for the 12 score-matmul + exp chunks."""
                chunks = []
                for j in range(8):
                    q0 = j * P
                    off = q0
                    while off < 1024:
                        n = min(512, 1024 - off)
                        def emit(j=j, q0=q0, off=off, n=n, first=(off == q0)):
                            ps = psmm.tile([P, 512], F32, name="mm", tag="mm")
                            nc.tensor.matmul(
                                ps[:, :n],
                                t2["k"][:, q0 : q0 + P],
                                t2["q"][:, off : off + n],
                                start=True,
                                stop=True,
                            )
                            nc.scalar.activation(
                                out=pt[:, PTOFF[j] + off - q0 : PTOFF[j] + off - q0 + n],
                                in_=ps[:, :n],
                                func=mybir.ActivationFunctionType.Exp,
                                scale=0.125,
                            )
                            if first:
                                nc.gpsimd.affine_select(
                                    out=pt[:, PTOFF[j] : PTOFF[j] + P],
                                    in_=pt[:, PTOFF[j] : PTOFF[j] + P],
                                    compare_op=mybir.AluOpType.is_ge,
                                    fill=0.0,
                                    base=0,
                                    pattern=[[1, P]],
                                    channel_multiplier=-1,
                                )
                        chunks.append(emit)
                        off += n
                return chunks

            def back_affines(h, pt, vo):
                pass

            def back_pv_quad(h, pt, vo, hs, quad):
                for iq in range(4):
                    i = quad * 4 + iq
                    pv = pspv.tile([P, DK + 1], F32, name="pv", tag="pv")
                    for j in range(i + 1):
                        nc.tensor.matmul(
                            pv[:],
                            pt[:, PTOFF[j] + (i - j) * P : PTOFF[j] + (i - j + 1) * P],
                            vo[:, j, : DK + 1],
                            start=(j == 0),
                            stop=(j == i),
                        )
                    # normalize straight out of PSUM on DVE (one hop
                    # shorter than staging through SBUF + Pool multiply)
                    rec = hsp.tile([P, 1], F32, name="rec", tag="rec")
                    nc.vector.reciprocal(rec[:], pv[:, DK : DK + 1])
                    nc.vector.tensor_scalar_mul(hs[:, i, :], pv[:, :DK], rec[:])

            def back_finish(h, hs):
                """Transpose to out_h^T, reorder-drain, scatter into opT."""
                psO = psoh.tile([64, 8, P], BF16, name="oht", tag="oht")
                for i in range(8):
                    nc.tensor.transpose(psO[:, i, :], hs[:, i, :], ident[:])
                # reorder drain: ohT[u, b*64 + i*8 + a'] = psO[u, i, a'*16 + b]
                ohT = ohtp.tile([64, 1024], BF16, name="ohT", tag="ohT")
                nc.vector.tensor_copy(
                    out=ohT[:].rearrange("u (b i a) -> u i a b", b=16, i=8, a=8),
                    in_=psO[:].rearrange("u i (a b) -> u i a b", a=8, b=16),
                )
                # two scatter DMAs into opT (by b parity)
                sview = ohT[:].rearrange("u (b1 b0 a) -> b0 u b1 a", b1=8, b0=2, a=64)
                for b0 in range(2):
                    eng = nc.sync if b0 == 0 else nc.scalar
                    eng.dma_start(
                        opT[b0 * 64 : (b0 + 1) * 64, :, h * 64 : (h + 1) * 64],
                        sview[b0],
                    )

            def p3_unit(blk, ch, wo_sb):
                ps = psmm.tile([P, 512], F32, name="mm", tag="mm")
                for pbt in range(8):
                    nc.tensor.matmul(
                        ps[:],
                        opT[:, pbt, blk * P : (blk + 1) * P],
                        wo_sb[:, pbt, ch * 512 : (ch + 1) * 512],
                        start=(pbt == 0),
                        stop=(pbt == 7),
                    )
                stg = outp.tile([P, 512], F32, name="ostg", tag="ostg")
                drain(stg[:], ps[:])
                eng = nc.sync if ch == 0 else nc.scalar
                eng.dma_start(
                    out_d.ap()[blk * P : (blk + 1) * P, ch * 512 : (ch + 1) * 512],
                    stg[:],
                )

            # ---------------- emission schedule ----------------
            xT0q = strip_loads("q", 0)
            wq_sb = load_w(wq_d)
            xT0k = strip_loads("k", 0)
            wk_sb = load_w(wk_d)
            strip_proj("q", 0, wq_sb, xT0q)
            xT0v = strip_loads("v", 0)
            wv_sb = load_w(wv_d)
            strip_proj("k", 0, wk_sb, xT0k)
            strip_proj("v", 0, wv_sb, xT0v)

            wo_sb = None
            s1 = {}
            prev = None
            blocks_done = 0

            def run_back(back):
                h_b, pt_b, vo_b = back
                hs_b = hsp.tile([P, 8, DK], BF16, name="hs", tag="hs")
                back_pv_quad(h_b, pt_b, vo_b, hs_b, 0)
                back_pv_quad(h_b, pt_b, vo_b, hs_b, 1)
                back_finish(h_b, hs_b)

            def hooks(h):
                nonlocal wo_sb
                if h == 1:
                    s1["xq"] = strip_loads("q", 1)
                elif h == 2:
                    s1["wq"] = load_w(wq_d)
                    s1["xk"] = strip_loads("k", 1)
                elif h == 3:
                    strip_proj("q", 1, s1.pop("wq"), s1.pop("xq"))
                    s1["wk"] = load_w(wk_d)
                    s1["xv"] = strip_loads("v", 1)
                    # wv one head early: SWDGE DMAs can stall ~12us behind
                    # unrelated queue semaphores, so give the load slack
                    # (legal: wq1's readers were just emitted)
                    s1["wv"] = load_w(wv_d)
                elif h == 4:
                    strip_proj("k", 1, s1.pop("wk"), s1.pop("xk"))
                    wo_sb = load_w(wo_d)
                elif h == 5:
                    strip_proj("v", 1, s1.pop("wv"), s1.pop("xv"))

            for h in range(H):
                hooks(h)
                pt, vo, t2 = head_front_a(h)
                sc = score_chunks(h, pt, t2)
                if prev is None:
                    for e in sc:
                        e()
                else:
                    # weave head h-1's PV/normalize/scatter between head h's
                    # score chunks so the PE never parks on a PSUM slot
                    # waiting for ACT's exp stream
                    h_b, pt_b, vo_b = prev
                    hs_b = hsp.tile([P, 8, DK], BF16, name="hs", tag="hs")
                    sc[0](); sc[1]()
                    back_pv_quad(h_b, pt_b, vo_b, hs_b, 0)
                    sc[2](); sc[3](); sc[4](); sc[5]()
                    back_pv_quad(h_b, pt_b, vo_b, hs_b, 1)
                    sc[6](); sc[7](); sc[8](); sc[9]()
                    back_finish(h_b, hs_b)
                    sc[10](); sc[11]()
                if prev is not None:
                    if h >= 7:
                        b_ready = (h - 2) // 2
                        while blocks_done < min(b_ready, 7):
                            p3_unit(blocks_done, 0, wo_sb)
                            p3_unit(blocks_done, 1, wo_sb)
                            blocks_done += 1
                prev = (h, pt, vo)
            run_back(prev)
            while blocks_done < 8:
                p3_unit(blocks_done, 0, wo_sb)
                p3_unit(blocks_done, 1, wo_sb)
                blocks_done += 1

    if not nc.is_finalized():
        nc.finalize()
    return nc


_nc_cache = {}


def _get_nc(mm_mode="bf16"):
    if mm_mode not in _nc_cache:
        _nc_cache[mm_mode] = _build_nc(mm_mode)
    return _nc_cache[mm_mode]


MM_MODE = "bf16"


def kernel(q, k, v, mask, w_q, w_k, w_v, w_o, _trace=False):
    q = np.ascontiguousarray(np.asarray(q, dtype=np.float32))
    k = np.ascontiguousarray(np.asarray(k, dtype=np.float32))
    v = np.ascontiguousarray(np.asarray(v, dtype=np.float32))
    w_q = np.ascontiguousarray(np.asarray(w_q, dtype=np.float32))
    w_k = np.ascontiguousarray(np.asarray(w_k, dtype=np.float32))
    w_v = np.ascontiguousarray(np.asarray(w_v, dtype=np.float32))
    w_o = np.ascontiguousarray(np.asarray(w_o, dtype=np.float32))

    nc = _get_nc()
    in_maps = [
        {
            "q": q[i],
            "k": k[i],
            "v": v[i],
            "w_q": w_q,
            "w_k": w_k,
            "w_v": w_v,
            "w_o": w_o,
        }
        for i in range(B)
    ]
    res = run_bass_kernel_spmd(
        nc, in_maps, core_ids=list(range(B)), trace=_trace
    )
    out = np.stack([r["out"] for r in res.results], axis=0)
    if _trace:
        kernel.last_exec_time_ns = res.exec_time_ns
        kernel.last_trace = res.instructions_and_trace
    return out


# revision 53
# speedup vs baseline: 1.0260x; 1.0260x over previous
"""Multi-head attention Trainium2 kernel (B=8,S=1024,D=1024,H=16,DK=64).

Data-parallel over batch: one batch element per NeuronCore (8 cores).
All matmuls in bf16 (1 PE cycle/row vs 4 for fp32); fp32 PSUM accumulation.
Fully SBUF-resident: no DRAM scratch round trips.

Per core:
  1. x (q/k/v) loaded with casting gpsimd DMAs (fp32->bf16), transposed to
     x^T via XBAR dma_start_transpose, then QP^T/KP^T/VP^T computed directly
     (weights stationary): qpT[p, j, s] = QP^T[j*128+p, s], in two s-strips
     of 512 so heads 0-7 can start after strip 0.
  2. torch-.view head split: head h reads only s in [h*64,(h+1)*64).
     Q_h^T[u, t] = QP^T[b*64+u, h*64+a] for t = a*16+b.  Per head, the
     u-contiguous [64, 1024] operand qT2 is built with 2 strided SBUF
     copies: even b from qpT[0:64], odd b from qpT2[0:64], where qpT2 is a
     partition-swapped twin (qpT2[0:64] = qpT[64:128]) made with one
     SBUF->SBUF DMA per strip.
  3. S^T[k, q] = K_h Q_h^T / 8 via 64-contraction matmuls (k on out
     partitions), exp on ACT into packed causal pt strips (bf16), causal
     diagonal via in-place affine_select.
  4. PV with V augmented by a ones column -> unnormalized out + sumexp in
     PSUM; normalized via DVE reciprocal + Pool multiply (walrus rejects
     divide and any Pool access to PSUM).  Head h's PV/normalize/scatter
     is woven between head h+1's score chunks (1-head pipeline skew).
  5. Per head: transpose hs -> out_h^T, reorder-drain to (b, a) layout,
     two scatter DMAs into OP^T; final projection OP^T x w_o from SBUF.
"""

import numpy as np

import concourse.bass as bass
import concourse.mybir as mybir
import concourse.tile as tile
from concourse import bacc
from concourse.bass_utils import run_bass_kernel_spmd
from concourse.masks import make_identity

B, S, D, H, DK = 8, 1024, 1024, 16, 64
P = 128
F32 = mybir.dt.float32
BF16 = mybir.dt.bfloat16

INTERLEAVE = False
SKEW = 1

# Fallback flags (flip if a feature fails to compile/execute)
USE_XBAR = True        # dma_start_transpose for x^T (else PE transposes)
USE_DIVIDE = True      # tensor_scalar divide (else reciprocal + mul)
USE_CAST_DMA = True    # gpsimd casting DMA loads (else hwdge + convert)

# packed causal pt strip offsets: strip j holds q in [j*128, 1024)
PTOFF = [j * 1024 - 64 * j * (j - 1) for j in range(9)]  # PTOFF[8] = 4608


def _build_nc(mm_mode: str = "bf16"):
    nc = bacc.Bacc(
        "TRN2",
        target_bir_lowering=False,
        debug=False,
        enable_asserts=False,
        num_devices=B,
    )

    q_d = nc.dram_tensor("q", [S, D], F32, kind="ExternalInput")
    k_d = nc.dram_tensor("k", [S, D], F32, kind="ExternalInput")
    v_d = nc.dram_tensor("v", [S, D], F32, kind="ExternalInput")
    wq_d = nc.dram_tensor("w_q", [D, D], F32, kind="ExternalInput")
    wk_d = nc.dram_tensor("w_k", [D, D], F32, kind="ExternalInput")
    wv_d = nc.dram_tensor("w_v", [D, D], F32, kind="ExternalInput")
    wo_d = nc.dram_tensor("w_o", [D, D], F32, kind="ExternalInput")
    out_d = nc.dram_tensor("out", [S, D], F32, kind="ExternalOutput")

    # enlarge the SWDGE descriptor ring: 512-desc casting loads otherwise
    # fill the default 1024-desc carveout and serialize behind transfers
    nc.dynamic_dma_scratch_size = 16384 * 8

    x_ds = {"q": q_d, "k": k_d, "v": v_d}
    w_ds = {"q": wq_d, "k": wk_d, "v": wv_d}

    with tile.TileContext(nc) as tc:
        with (
            tc.tile_pool(name="consts", bufs=1) as consts,
            tc.tile_pool(name="wp", bufs=2) as wp,
            tc.tile_pool(name="xbfp", bufs=2) as xbfp,
            tc.tile_pool(name="xtp", bufs=2) as xtp,
            tc.tile_pool(name="projp", bufs=1) as projp,
            tc.tile_pool(name="proj2p", bufs=1) as proj2p,
            tc.tile_pool(name="optp", bufs=1) as optp,
            tc.tile_pool(name="t2p", bufs=2) as t2p,
            tc.tile_pool(name="ptp", bufs=2) as ptp,
            tc.tile_pool(name="vop", bufs=2) as vop,
            tc.tile_pool(name="hsp", bufs=2) as hsp,
            tc.tile_pool(name="ohtp", bufs=1) as ohtp,
            tc.tile_pool(name="outp", bufs=1) as outp,
            tc.tile_pool(name="psmm", bufs=3, space="PSUM") as psmm,
            tc.tile_pool(name="pspv", bufs=3, space="PSUM") as pspv,
            tc.tile_pool(name="psoh", bufs=2, space="PSUM") as psoh,
        ):
            ident = consts.tile([P, P], BF16, tag="ident")
            make_identity(nc, ident[:])

            # persistent vT2 [80, 1024]: rows 0:64 rewritten per head (V_h^T),
            # rows 64:80 stay 1.0 so the vo XBAR transpose lands the sumexp
            # ones column at position 64 of each 80-wide row.
            vT2 = consts.tile([80, 1024], BF16, tag="vT2")
            nc.gpsimd.memset(vT2[64:80, :], 1.0)

            # persistent projection outputs (transposed) + swapped twins
            pT = {t: projp.tile([P, 8, 1024], BF16, name=f"{t}pT", tag=f"{t}pT") for t in "qkv"}
            pT2 = {t: proj2p.tile([64, 8, 1024], BF16, name=f"{t}pT2", tag=f"{t}pT2") for t in "qkv"}
            opT = optp.tile([P, 8, 1024], BF16, tag="opT")

            drain_flip = [0]

            def drain(out_ap, in_ap, act_ok=False):
                # PSUM->SBUF drains (Pool has no PSUM access path).  ACT
                # copies are only used while ACT is otherwise idle (strip 0):
                # during phase 2 a copy whose matmul is late would
                # head-of-line-block the exp stream on ACT's in-order queue.
                drain_flip[0] += 1
                if act_ok and drain_flip[0] % 2 == 1:
                    nc.scalar.copy(out=out_ap, in_=in_ap)
                else:
                    nc.vector.tensor_copy(out=out_ap, in_=in_ap)

            def load_w(w_d, halves=2):
                # casting DMAs per weight (SWDGE desc-gen on Pool)
                w_sb = wp.tile([P, 8, 1024], BF16, tag="w")
                wsrc = w_d.ap().rearrange("(j p) c -> p j c", p=P)
                csz = 1024 // halves
                for chalf in range(halves):
                    c0 = chalf * csz
                    nc.gpsimd.dma_start(
                        w_sb[:, :, c0 : c0 + csz], wsrc[:, :, c0 : c0 + csz]
                    )
                return w_sb

            def strip_loads(t, st):
                """Load x strip as bf16 and XBAR-transpose into an xT tile."""
                s0 = st * 512
                x_d = x_ds[t]
                xT = xtp.tile([P, 8, 512], BF16, tag="xT")
                for half in range(2):
                    r0 = s0 + half * 256
                    xbf = xbfp.tile([P, 2, 1024], BF16, tag="xbf")
                    nc.gpsimd.dma_start(
                        xbf[:],
                        x_d.ap()[r0 : r0 + 256, :].rearrange(
                            "(s2 p) c -> p s2 c", p=P
                        ),
                    )
                    for s2 in range(2):
                        # one batched XBAR per x-tile: out[p, j, f] =
                        # x^T[j*128+p, f] (verified on HW).  All strip/store
                        # DMAs ride sync; per-head DMAs ride scalar, so a
                        # strip XBAR never queues behind head traffic (its
                        # completion gates the next xbf load via slot WAR).
                        s_lo = half * 256 + s2 * P
                        eng = nc.sync
                        eng.dma_start_transpose(
                            xT[:, :, s_lo : s_lo + P], xbf[:, s2, :]
                        )
                return xT

            def proj_unit(t, st, w_sb, xT, j):
                """One projection j-group (8 matmuls + drain); the last unit
                also emits the partition-swap twin DMA."""
                s0 = st * 512
                ps = psmm.tile([P, 512], F32, name="mm", tag="mm")
                for dit in range(8):
                    nc.tensor.matmul(
                        ps[:],
                        w_sb[:, dit, j * P : (j + 1) * P],
                        xT[:, dit, :],
                        start=(dit == 0),
                        stop=(dit == 7),
                    )
                drain(pT[t][:, j, s0 : s0 + 512], ps[:], act_ok=(st == 0))
                if j == 7:
                    # partition-swap twin: pT2[0:64] = pT[64:128]
                    eng = nc.sync
                    eng.dma_start(
                        pT2[t][0:64, :, s0 : s0 + 512],
                        pT[t][64:P, :, s0 : s0 + 512],
                    )

            def strip_proj(t, st, w_sb, xT):
                for j in range(8):
                    proj_unit(t, st, w_sb, xT, j)

            def head_front_a(h):
                """qT2/kT2/vT2 copies and the vo XBAR for head h."""
                t2 = {}
                for idx, t in enumerate("qkv"):
                    if t == "v":
                        dstT = vT2
                    else:
                        dstT = t2p.tile(
                            [64, 1024], BF16, name=f"{t}T2", tag=f"{t}T2"
                        )
                    t2[t] = dstT
                    dview = dstT[0:64, :].rearrange(
                        "u (a b1 b0) -> b0 u b1 a", a=64, b1=8, b0=2
                    )
                    for b0 in range(2):
                        src = (pT[t] if b0 == 0 else pT2[t])[
                            0:64, :, h * 64 : (h + 1) * 64
                        ]
                        eng = nc.gpsimd if (idx * 2 + b0) % 2 == 0 else nc.vector
                        eng.tensor_copy(out=dview[b0], in_=src)

                # vo[p, j, u] = V_h[j*128+p, u] for u<64, 1.0 at u=64 (sumexp)
                vo = vop.tile([P, 8, 80], BF16, name="vo", tag="vo")
                nc.scalar.dma_start_transpose(vo[:], vT2[:])
                pt = ptp.tile([P, PTOFF[8]], BF16, name="pt", tag="pt")
                return pt, vo, t2

            def score_chunks(h, pt, t2):
                """Emitter thunks for the 12 score-matmul + exp chunks."""
                chunks = []
                for j in range(8):
                    q0 = j * P
                    off = q0
                    while off < 1024:
                        n = min(512, 1024 - off)
                        def emit(j=j, q0=q0, off=off, n=n, first=(off == q0)):
                            ps = psmm.tile([P, 512], F32, name="mm", tag="mm")
                            nc.tensor.matmul(
                                ps[:, :n],
                                t2["k"][:, q0 : q0 + P],
                                t2["q"][:, off : off + n],
                                start=True,
                                stop=True,
                            )
                            nc.scalar.activation(
                                out=pt[:, PTOFF[j] + off - q0 : PTOFF[j] + off - q0 + n],
                                in_=ps[:, :n],
                                func=mybir.ActivationFunctionType.Exp,
                                scale=0.125,
                            )
                            if first:
                                nc.gpsimd.affine_select(
                                    out=pt[:, PTOFF[j] : PTOFF[j] + P],
                                    in_=pt[:, PTOFF[j] : PTOFF[j] + P],
                                    compare_op=mybir.AluOpType.is_ge,
                                    fill=0.0,
                                    base=0,
                                    pattern=[[1, P]],
                                    channel_multiplier=-1,
                                )
                        chunks.append(emit)
                        off += n
                return chunks

            def back_affines(h, pt, vo):
                pass

            def back_pv_quad(h, pt, vo, hs, quad):
                for iq in range(4):
                    i = quad * 4 + iq
                    pv = pspv.tile([P, DK + 1], F32, name="pv", tag="pv")
                    for j in range(i + 1):
                        nc.tensor.matmul(
                            pv[:],
                            pt[:, PTOFF[j] + (i - j) * P : PTOFF[j] + (i - j + 1) * P],
                            vo[:, j, : DK + 1],
                            start=(j == 0),
                            stop=(j == i),
                        )
                    # normalize straight out of PSUM on DVE (one hop
                    # shorter than staging through SBUF + Pool multiply)
                    rec = hsp.tile([P, 1], F32, name="rec", tag="rec")
                    nc.vector.reciprocal(rec[:], pv[:, DK : DK + 1])
                    nc.vector.tensor_scalar_mul(hs[:, i, :], pv[:, :DK], rec[:])

            def back_finish(h, hs):
                """Transpose to out_h^T, reorder-drain, scatter into opT."""
                psO = psoh.tile([64, 8, P], BF16, name="oht", tag="oht")
                for i in range(8):
                    nc.tensor.transpose(psO[:, i, :], hs[:, i, :], ident[:])
                # reorder drain: ohT[u, b*64 + i*8 + a'] = psO[u, i, a'*16 + b]
                ohT = ohtp.tile([64, 1024], BF16, name="ohT", tag="ohT")
                nc.vector.tensor_copy(
                    out=ohT[:].rearrange("u (b i a) -> u i a b", b=16, i=8, a=8),
                    in_=psO[:].rearrange("u i (a b) -> u i a b", a=8, b=16),
                )
                # two scatter DMAs into opT (by b parity)
                sview = ohT[:].rearrange("u (b1 b0 a) -> b0 u b1 a", b1=8, b0=2, a=64)
                for b0 in range(2):
                    eng = nc.scalar
                    eng.dma_start(
                        opT[b0 * 64 : (b0 + 1) * 64, :, h * 64 : (h + 1) * 64],
                        sview[b0],
                    )

            def p3_unit(blk, ch, wo_sb):
                ps = psmm.tile([P, 512], F32, name="mm", tag="mm")
                for pbt in range(8):
                    nc.tensor.matmul(
                        ps[:],
                        opT[:, pbt, blk * P : (blk + 1) * P],
                        wo_sb[:, pbt, ch * 512 : (ch + 1) * 512],
                        start=(pbt == 0),
                        stop=(pbt == 7),
                    )
                stg = outp.tile([P, 512], F32, name="ostg", tag="ostg")
                drain(stg[:], ps[:])
                eng = nc.sync
                eng.dma_start(
                    out_d.ap()[blk * P : (blk + 1) * P, ch * 512 : (ch + 1) * 512],
                    stg[:],
                )

            # ---------------- emission schedule ----------------
            xT0q = strip_loads("q", 0)
            wq_sb = load_w(wq_d)
            xT0k = strip_loads("k", 0)
            wk_sb = load_w(wk_d)
            strip_proj("q", 0, wq_sb, xT0q)
            xT0v = strip_loads("v", 0)
            wv_sb = load_w(wv_d)
            strip_proj("k", 0, wk_sb, xT0k)
            strip_proj("v", 0, wv_sb, xT0v)

            wo_sb = None
            s1 = {}
            prev = None
            blocks_done = 0

            def run_back(back):
                h_b, pt_b, vo_b = back
                hs_b = hsp.tile([P, 8, DK], BF16, name="hs", tag="hs")
                back_pv_quad(h_b, pt_b, vo_b, hs_b, 0)
                back_pv_quad(h_b, pt_b, vo_b, hs_b, 1)
                back_finish(h_b, hs_b)

            def hooks(h):
                nonlocal wo_sb
                if h == 1:
                    s1["xq"] = strip_loads("q", 1)
                elif h == 2:
                    s1["wq"] = load_w(wq_d)
                    s1["xk"] = strip_loads("k", 1)
                elif h == 3:
                    strip_proj("q", 1, s1.pop("wq"), s1.pop("xq"))
                    s1["wk"] = load_w(wk_d)
                    s1["xv"] = strip_loads("v", 1)
                    # wv one head early: SWDGE DMAs can stall ~12us behind
                    # unrelated queue semaphores, so give the load slack
                    # (legal: wq1's readers were just emitted)
                    s1["wv"] = load_w(wv_d)
                elif h == 4:
                    strip_proj("k", 1, s1.pop("wk"), s1.pop("xk"))
                    wo_sb = load_w(wo_d)
                elif h == 5:
                    strip_proj("v", 1, s1.pop("wv"), s1.pop("xv"))

            for h in range(H):
                hooks(h)
                pt, vo, t2 = head_front_a(h)
                sc = score_chunks(h, pt, t2)
                if prev is None:
                    for e in sc:
                        e()
                else:
                    # weave head h-1's PV/normalize/scatter between head h's
                    # score chunks so the PE never parks on a PSUM slot
                    # waiting for ACT's exp stream
                    h_b, pt_b, vo_b = prev
                    hs_b = hsp.tile([P, 8, DK], BF16, name="hs", tag="hs")
                    sc[0](); sc[1]()
                    back_pv_quad(h_b, pt_b, vo_b, hs_b, 0)
                    sc[2](); sc[3](); sc[4](); sc[5]()
                    back_pv_quad(h_b, pt_b, vo_b, hs_b, 1)
                    sc[6](); sc[7](); sc[8](); sc[9]()
                    back_finish(h_b, hs_b)
                    sc[10](); sc[11]()
                if prev is not None:
                    if h >= 7:
                        b_ready = (h - 2) // 2
                        while blocks_done < min(b_ready, 7):
                            p3_unit(blocks_done, 0, wo_sb)
                            p3_unit(blocks_done, 1, wo_sb)
                            blocks_done += 1
                prev = (h, pt, vo)
            run_back(prev)
            while blocks_done < 8:
                p3_unit(blocks_done, 0, wo_sb)
                p3_unit(blocks_done, 1, wo_sb)
                blocks_done += 1

    if not nc.is_finalized():
        nc.finalize()
    return nc


_nc_cache = {}


def _get_nc(mm_mode="bf16"):
    if mm_mode not in _nc_cache:
        _nc_cache[mm_mode] = _build_nc(mm_mode)
    return _nc_cache[mm_mode]


MM_MODE = "bf16"


def kernel(q, k, v, mask, w_q, w_k, w_v, w_o, _trace=False):
    q = np.ascontiguousarray(np.asarray(q, dtype=np.float32))
    k = np.ascontiguousarray(np.asarray(k, dtype=np.float32))
    v = np.ascontiguousarray(np.asarray(v, dtype=np.float32))
    w_q = np.ascontiguousarray(np.asarray(w_q, dtype=np.float32))
    w_k = np.ascontiguousarray(np.asarray(w_k, dtype=np.float32))
    w_v = np.ascontiguousarray(np.asarray(w_v, dtype=np.float32))
    w_o = np.ascontiguousarray(np.asarray(w_o, dtype=np.float32))

    nc = _get_nc()
    in_maps = [
        {
            "q": q[i],
            "k": k[i],
            "v": v[i],
            "w_q": w_q,
            "w_k": w_k,
            "w_v": w_v,
            "w_o": w_o,
        }
        for i in range(B)
    ]
    res = run_bass_kernel_spmd(
        nc, in_maps, core_ids=list(range(B)), trace=_trace
    )
    out = np.stack([r["out"] for r in res.results], axis=0)
    if _trace:
        kernel.last_exec_time_ns = res.exec_time_ns
        kernel.last_trace = res.instructions_and_trace
    return out


# revision 54
# speedup vs baseline: 1.1462x; 1.1172x over previous
"""Multi-head attention Trainium2 kernel (B=8,S=1024,D=1024,H=16,DK=64).

Data-parallel over batch: one batch element per NeuronCore (8 cores).
All matmuls in bf16 (1 PE cycle/row vs 4 for fp32); fp32 PSUM accumulation.
Fully SBUF-resident: no DRAM scratch round trips.

Per core:
  1. x (q/k/v) loaded with casting gpsimd DMAs (fp32->bf16), transposed to
     x^T via XBAR dma_start_transpose, then QP^T/KP^T/VP^T computed directly
     (weights stationary): qpT[p, j, s] = QP^T[j*128+p, s], in two s-strips
     of 512 so heads 0-7 can start after strip 0.
  2. torch-.view head split: head h reads only s in [h*64,(h+1)*64).
     Q_h^T[u, t] = QP^T[b*64+u, h*64+a] for t = a*16+b.  Per head, the
     u-contiguous [64, 1024] operand qT2 is built with 2 strided SBUF
     copies: even b from qpT[0:64], odd b from qpT2[0:64], where qpT2 is a
     partition-swapped twin (qpT2[0:64] = qpT[64:128]) made with one
     SBUF->SBUF DMA per strip.
  3. S^T[k, q] = K_h Q_h^T / 8 via 64-contraction matmuls (k on out
     partitions), exp on ACT into packed causal pt strips (bf16), causal
     diagonal via in-place affine_select.
  4. PV with V augmented by a ones column -> unnormalized out + sumexp in
     PSUM; normalized via DVE reciprocal + Pool multiply (walrus rejects
     divide and any Pool access to PSUM).  Head h's PV/normalize/scatter
     is woven between head h+1's score chunks (1-head pipeline skew).
  5. Per head: transpose hs -> out_h^T, reorder-drain to (b, a) layout,
     two scatter DMAs into OP^T; final projection OP^T x w_o from SBUF.
"""

import numpy as np

import concourse.bass as bass
import concourse.mybir as mybir
import concourse.tile as tile
from concourse import bacc
from concourse.bass_utils import run_bass_kernel_spmd
from concourse.masks import make_identity

B, S, D, H, DK = 8, 1024, 1024, 16, 64
P = 128
F32 = mybir.dt.float32
BF16 = mybir.dt.bfloat16

INTERLEAVE = False
SKEW = 1

# Fallback flags (flip if a feature fails to compile/execute)
USE_XBAR = True        # dma_start_transpose for x^T (else PE transposes)
USE_DIVIDE = True      # tensor_scalar divide (else reciprocal + mul)
USE_CAST_DMA = True    # gpsimd casting DMA loads (else hwdge + convert)

# packed causal pt strip offsets: strip j holds q in [j*128, 1024)
PTOFF = [j * 1024 - 64 * j * (j - 1) for j in range(9)]  # PTOFF[8] = 4608


def _build_nc(mm_mode: str = "bf16"):
    nc = bacc.Bacc(
        "TRN2",
        target_bir_lowering=False,
        debug=False,
        enable_asserts=False,
        num_devices=B,
    )

    q_d = nc.dram_tensor("q", [S, D], F32, kind="ExternalInput")
    k_d = nc.dram_tensor("k", [S, D], F32, kind="ExternalInput")
    v_d = nc.dram_tensor("v", [S, D], F32, kind="ExternalInput")
    wq_d = nc.dram_tensor("w_q", [D, D], F32, kind="ExternalInput")
    wk_d = nc.dram_tensor("w_k", [D, D], F32, kind="ExternalInput")
    wv_d = nc.dram_tensor("w_v", [D, D], F32, kind="ExternalInput")
    wo_d = nc.dram_tensor("w_o", [D, D], F32, kind="ExternalInput")
    out_d = nc.dram_tensor("out", [S, D], F32, kind="ExternalOutput")

    # enlarge the SWDGE descriptor ring: 512-desc casting loads otherwise
    # fill the default 1024-desc carveout and serialize behind transfers
    nc.dynamic_dma_scratch_size = 16384 * 8

    x_ds = {"q": q_d, "k": k_d, "v": v_d}
    w_ds = {"q": wq_d, "k": wk_d, "v": wv_d}

    with tile.TileContext(nc) as tc:
        with (
            tc.tile_pool(name="consts", bufs=1) as consts,
            tc.tile_pool(name="wp", bufs=2) as wp,
            tc.tile_pool(name="xbfp", bufs=2) as xbfp,
            tc.tile_pool(name="xtp", bufs=2) as xtp,
            tc.tile_pool(name="projp", bufs=1) as projp,
            tc.tile_pool(name="proj2p", bufs=1) as proj2p,
            tc.tile_pool(name="optp", bufs=1) as optp,
            tc.tile_pool(name="t2p", bufs=2) as t2p,
            tc.tile_pool(name="ptp", bufs=2) as ptp,
            tc.tile_pool(name="vop", bufs=2) as vop,
            tc.tile_pool(name="hsp", bufs=2) as hsp,
            tc.tile_pool(name="ohtp", bufs=1) as ohtp,
            tc.tile_pool(name="outp", bufs=1) as outp,
            tc.tile_pool(name="psmm", bufs=3, space="PSUM") as psmm,
            tc.tile_pool(name="pspv", bufs=3, space="PSUM") as pspv,
            tc.tile_pool(name="psoh", bufs=2, space="PSUM") as psoh,
        ):
            ident = consts.tile([P, P], BF16, tag="ident")
            make_identity(nc, ident[:])

            # persistent vT2 [80, 1024]: rows 0:64 rewritten per head (V_h^T),
            # rows 64:80 stay 1.0 so the vo XBAR transpose lands the sumexp
            # ones column at position 64 of each 80-wide row.
            vT2 = consts.tile([80, 1024], BF16, tag="vT2")
            nc.gpsimd.memset(vT2[64:80, :], 1.0)

            # persistent projection outputs (transposed) + swapped twins
            pT = {t: projp.tile([P, 8, 1024], BF16, name=f"{t}pT", tag=f"{t}pT") for t in "qkv"}
            pT2 = {t: proj2p.tile([64, 8, 1024], BF16, name=f"{t}pT2", tag=f"{t}pT2") for t in "qkv"}
            opT = optp.tile([P, 8, 1024], BF16, tag="opT")

            drain_flip = [0]

            def drain(out_ap, in_ap, act_ok=False):
                # PSUM->SBUF drains (Pool has no PSUM access path).  ACT
                # copies are only used while ACT is otherwise idle (strip 0):
                # during phase 2 a copy whose matmul is late would
                # head-of-line-block the exp stream on ACT's in-order queue.
                drain_flip[0] += 1
                if act_ok and drain_flip[0] % 2 == 1:
                    nc.scalar.copy(out=out_ap, in_=in_ap)
                else:
                    nc.vector.tensor_copy(out=out_ap, in_=in_ap)

            def load_w(w_d, halves=2):
                # casting DMAs per weight (SWDGE desc-gen on Pool)
                w_sb = wp.tile([P, 8, 1024], BF16, tag="w")
                wsrc = w_d.ap().rearrange("(j p) c -> p j c", p=P)
                csz = 1024 // halves
                for chalf in range(halves):
                    c0 = chalf * csz
                    nc.gpsimd.dma_start(
                        w_sb[:, :, c0 : c0 + csz], wsrc[:, :, c0 : c0 + csz]
                    )
                return w_sb

            def strip_loads(t, st):
                """Load x strip as bf16 and XBAR-transpose into an xT tile."""
                s0 = st * 512
                x_d = x_ds[t]
                xT = xtp.tile([P, 8, 512], BF16, tag="xT")
                for half in range(2):
                    r0 = s0 + half * 256
                    xbf = xbfp.tile([P, 2, 1024], BF16, tag="xbf")
                    nc.gpsimd.dma_start(
                        xbf[:],
                        x_d.ap()[r0 : r0 + 256, :].rearrange(
                            "(s2 p) c -> p s2 c", p=P
                        ),
                    )
                    for s2 in range(2):
                        # one batched XBAR per x-tile: out[p, j, f] =
                        # x^T[j*128+p, f] (verified on HW).  All strip/store
                        # DMAs ride sync; per-head DMAs ride scalar, so a
                        # strip XBAR never queues behind head traffic (its
                        # completion gates the next xbf load via slot WAR).
                        s_lo = half * 256 + s2 * P
                        eng = nc.sync
                        eng.dma_start_transpose(
                            xT[:, :, s_lo : s_lo + P], xbf[:, s2, :]
                        )
                return xT

            def proj_unit(t, st, w_sb, xT, j):
                """One projection j-group (8 matmuls + drain); the last unit
                also emits the partition-swap twin DMA."""
                s0 = st * 512
                ps = psmm.tile([P, 512], F32, name="mm", tag="mm")
                for dit in range(8):
                    nc.tensor.matmul(
                        ps[:],
                        w_sb[:, dit, j * P : (j + 1) * P],
                        xT[:, dit, :],
                        start=(dit == 0),
                        stop=(dit == 7),
                    )
                drain(pT[t][:, j, s0 : s0 + 512], ps[:], act_ok=(st == 0))
                if j == 7:
                    # partition-swap twin: pT2[0:64] = pT[64:128]
                    eng = nc.sync
                    eng.dma_start(
                        pT2[t][0:64, :, s0 : s0 + 512],
                        pT[t][64:P, :, s0 : s0 + 512],
                    )

            def strip_proj(t, st, w_sb, xT):
                for j in range(8):
                    proj_unit(t, st, w_sb, xT, j)

            def head_front_a(h):
                """qT2/kT2/vT2 copies and the vo XBAR for head h."""
                t2 = {}
                for idx, t in enumerate("qkv"):
                    if t == "v":
                        dstT = vT2
                    else:
                        dstT = t2p.tile(
                            [64, 1024], BF16, name=f"{t}T2", tag=f"{t}T2"
                        )
                    t2[t] = dstT
                    dview = dstT[0:64, :].rearrange(
                        "u (a b1 b0) -> b0 u b1 a", a=64, b1=8, b0=2
                    )
                    for b0 in range(2):
                        src = (pT[t] if b0 == 0 else pT2[t])[
                            0:64, :, h * 64 : (h + 1) * 64
                        ]
                        eng = nc.gpsimd if (idx * 2 + b0) % 2 == 0 else nc.vector
                        eng.tensor_copy(out=dview[b0], in_=src)

                # vo[p, j, u] = V_h[j*128+p, u] for u<64, 1.0 at u=64 (sumexp)
                vo = vop.tile([P, 8, 80], BF16, name="vo", tag="vo")
                nc.scalar.dma_start_transpose(vo[:], vT2[:])
                pt = ptp.tile([P, PTOFF[8]], BF16, name="pt", tag="pt")
                return pt, vo, t2

            def score_chunks(h, pt, t2):
                """Emitter thunks for the 12 score-matmul + exp chunks."""
                chunks = []
                for j in range(8):
                    q0 = j * P
                    off = q0
                    while off < 1024:
                        n = min(512, 1024 - off)
                        def emit(j=j, q0=q0, off=off, n=n, first=(off == q0)):
                            ps = psmm.tile([P, 512], F32, name="mm", tag="mm")
                            nc.tensor.matmul(
                                ps[:, :n],
                                t2["k"][:, q0 : q0 + P],
                                t2["q"][:, off : off + n],
                                start=True,
                                stop=True,
                            )
                            nc.scalar.activation(
                                out=pt[:, PTOFF[j] + off - q0 : PTOFF[j] + off - q0 + n],
                                in_=ps[:, :n],
                                func=mybir.ActivationFunctionType.Exp,
                                scale=0.125,
                            )
                            if first:
                                nc.gpsimd.affine_select(
                                    out=pt[:, PTOFF[j] : PTOFF[j] + P],
                                    in_=pt[:, PTOFF[j] : PTOFF[j] + P],
                                    compare_op=mybir.AluOpType.is_ge,
                                    fill=0.0,
                                    base=0,
                                    pattern=[[1, P]],
                                    channel_multiplier=-1,
                                )
                        chunks.append(emit)
                        off += n
                return chunks

            def back_affines(h, pt, vo):
                pass

            def back_pv_quad(h, pt, vo, hs, quad):
                for iq in range(4):
                    i = quad * 4 + iq
                    pv = pspv.tile([P, DK + 1], F32, name="pv", tag="pv")
                    for j in range(i + 1):
                        nc.tensor.matmul(
                            pv[:],
                            pt[:, PTOFF[j] + (i - j) * P : PTOFF[j] + (i - j + 1) * P],
                            vo[:, j, : DK + 1],
                            start=(j == 0),
                            stop=(j == i),
                        )
                    # normalize straight out of PSUM on DVE (one hop
                    # shorter than staging through SBUF + Pool multiply)
                    rec = hsp.tile([P, 1], F32, name="rec", tag="rec")
                    nc.vector.reciprocal(rec[:], pv[:, DK : DK + 1])
                    nc.vector.tensor_scalar_mul(hs[:, i, :], pv[:, :DK], rec[:])

            def back_finish(h, hs):
                """Transpose to out_h^T, reorder-drain, scatter into opT."""
                psO = psoh.tile([64, 8, P], BF16, name="oht", tag="oht")
                for i in range(8):
                    nc.tensor.transpose(psO[:, i, :], hs[:, i, :], ident[:])
                # reorder drain: ohT[u, b*64 + i*8 + a'] = psO[u, i, a'*16 + b]
                ohT = ohtp.tile([64, 1024], BF16, name="ohT", tag="ohT")
                nc.vector.tensor_copy(
                    out=ohT[:].rearrange("u (b i a) -> u i a b", b=16, i=8, a=8),
                    in_=psO[:].rearrange("u i (a b) -> u i a b", a=8, b=16),
                )
                # two scatter DMAs into opT (by b parity)
                sview = ohT[:].rearrange("u (b1 b0 a) -> b0 u b1 a", b1=8, b0=2, a=64)
                for b0 in range(2):
                    eng = nc.scalar
                    eng.dma_start(
                        opT[b0 * 64 : (b0 + 1) * 64, :, h * 64 : (h + 1) * 64],
                        sview[b0],
                    )

            def p3_unit(blk, ch, wo_sb):
                ps = psmm.tile([P, 512], F32, name="mm", tag="mm")
                for pbt in range(8):
                    nc.tensor.matmul(
                        ps[:],
                        opT[:, pbt, blk * P : (blk + 1) * P],
                        wo_sb[:, pbt, ch * 512 : (ch + 1) * 512],
                        start=(pbt == 0),
                        stop=(pbt == 7),
                    )
                stg = outp.tile([P, 512], F32, name="ostg", tag="ostg")
                drain(stg[:], ps[:])
                eng = nc.sync
                eng.dma_start(
                    out_d.ap()[blk * P : (blk + 1) * P, ch * 512 : (ch + 1) * 512],
                    stg[:],
                )

            # ---------------- emission schedule ----------------
            # All six projection strips run up-front with loads interleaved
            # (each w loaded ONCE, reused by both strips: halves the weight
            # DMA), then all 16 heads run back-to-back with the output
            # projection as PE filler.  This un-bunches the DMA device,
            # which previously stalled the PE ~25us around the mid-phase
            # strip-1 reloads.
            xTq0 = strip_loads("q", 0)
            wq_sb = load_w(wq_d)
            xTq1 = strip_loads("q", 1)
            strip_proj("q", 0, wq_sb, xTq0)
            xTk0 = strip_loads("k", 0)
            strip_proj("q", 1, wq_sb, xTq1)
            wk_sb = load_w(wk_d)
            xTk1 = strip_loads("k", 1)
            strip_proj("k", 0, wk_sb, xTk0)
            wv_sb = load_w(wv_d)          # slot of wq: its readers are done
            xTv0 = strip_loads("v", 0)
            strip_proj("k", 1, wk_sb, xTk1)
            xTv1 = strip_loads("v", 1)
            strip_proj("v", 0, wv_sb, xTv0)
            wo_sb = load_w(wo_d)          # slot of wk: its readers are done
            strip_proj("v", 1, wv_sb, xTv1)

            prev = None
            blocks_done = 0
            for h in range(H):
                pt, vo, t2 = head_front_a(h)
                sc = score_chunks(h, pt, t2)
                if prev is None:
                    for e in sc:
                        e()
                else:
                    h_b, pt_b, vo_b = prev
                    hs_b = hsp.tile([P, 8, DK], BF16, name="hs", tag="hs")
                    sc[0](); sc[1]()
                    back_pv_quad(h_b, pt_b, vo_b, hs_b, 0)
                    sc[2](); sc[3](); sc[4](); sc[5]()
                    back_pv_quad(h_b, pt_b, vo_b, hs_b, 1)
                    sc[6](); sc[7](); sc[8](); sc[9]()
                    back_finish(h_b, hs_b)
                    sc[10](); sc[11]()
                if prev is not None:
                    if h >= 3:
                        b_ready = (h - 2) // 2
                        while blocks_done < min(b_ready, 7):
                            p3_unit(blocks_done, 0, wo_sb)
                            p3_unit(blocks_done, 1, wo_sb)
                            blocks_done += 1
                prev = (h, pt, vo)
            h_b, pt_b, vo_b = prev
            hs_b = hsp.tile([P, 8, DK], BF16, name="hs", tag="hs")
            back_pv_quad(h_b, pt_b, vo_b, hs_b, 0)
            back_pv_quad(h_b, pt_b, vo_b, hs_b, 1)
            back_finish(h_b, hs_b)
            while blocks_done < 8:
                p3_unit(blocks_done, 0, wo_sb)
                p3_unit(blocks_done, 1, wo_sb)
                blocks_done += 1

    if not nc.is_finalized():
        nc.finalize()
    return nc


_nc_cache = {}


def _get_nc(mm_mode="bf16"):
    if mm_mode not in _nc_cache:
        _nc_cache[mm_mode] = _build_nc(mm_mode)
    return _nc_cache[mm_mode]


MM_MODE = "bf16"


def kernel(q, k, v, mask, w_q, w_k, w_v, w_o, _trace=False):
    q = np.ascontiguousarray(np.asarray(q, dtype=np.float32))
    k = np.ascontiguousarray(np.asarray(k, dtype=np.float32))
    v = np.ascontiguousarray(np.asarray(v, dtype=np.float32))
    w_q = np.ascontiguousarray(np.asarray(w_q, dtype=np.float32))
    w_k = np.ascontiguousarray(np.asarray(w_k, dtype=np.float32))
    w_v = np.ascontiguousarray(np.asarray(w_v, dtype=np.float32))
    w_o = np.ascontiguousarray(np.asarray(w_o, dtype=np.float32))

    nc = _get_nc()
    in_maps = [
        {
            "q": q[i],
            "k": k[i],
            "v": v[i],
            "w_q": w_q,
            "w_k": w_k,
            "w_v": w_v,
            "w_o": w_o,
        }
        for i in range(B)
    ]
    res = run_bass_kernel_spmd(
        nc, in_maps, core_ids=list(range(B)), trace=_trace
    )
    out = np.stack([r["out"] for r in res.results], axis=0)
    if _trace:
        kernel.last_exec_time_ns = res.exec_time_ns
        kernel.last_trace = res.instructions_and_trace
    return out


# revision 55
# speedup vs baseline: 1.1739x; 1.0241x over previous
"""Multi-head attention Trainium2 kernel (B=8,S=1024,D=1024,H=16,DK=64).

Data-parallel over batch: one batch element per NeuronCore (8 cores).
All matmuls in bf16 (1 PE cycle/row vs 4 for fp32); fp32 PSUM accumulation.
Fully SBUF-resident: no DRAM scratch round trips.

Per core:
  1. x (q/k/v) loaded with casting gpsimd DMAs (fp32->bf16), transposed to
     x^T via XBAR dma_start_transpose, then QP^T/KP^T/VP^T computed directly
     (weights stationary): qpT[p, j, s] = QP^T[j*128+p, s], in two s-strips
     of 512 so heads 0-7 can start after strip 0.
  2. torch-.view head split: head h reads only s in [h*64,(h+1)*64).
     Q_h^T[u, t] = QP^T[b*64+u, h*64+a] for t = a*16+b.  Per head, the
     u-contiguous [64, 1024] operand qT2 is built with 2 strided SBUF
     copies: even b from qpT[0:64], odd b from qpT2[0:64], where qpT2 is a
     partition-swapped twin (qpT2[0:64] = qpT[64:128]) made with one
     SBUF->SBUF DMA per strip.
  3. S^T[k, q] = K_h Q_h^T / 8 via 64-contraction matmuls (k on out
     partitions), exp on ACT into packed causal pt strips (bf16), causal
     diagonal via in-place affine_select.
  4. PV with V augmented by a ones column -> unnormalized out + sumexp in
     PSUM; normalized via DVE reciprocal + Pool multiply (walrus rejects
     divide and any Pool access to PSUM).  Head h's PV/normalize/scatter
     is woven between head h+1's score chunks (1-head pipeline skew).
  5. Per head: transpose hs -> out_h^T, reorder-drain to (b, a) layout,
     two scatter DMAs into OP^T; final projection OP^T x w_o from SBUF.
"""

import numpy as np

import concourse.bass as bass
import concourse.mybir as mybir
import concourse.tile as tile
from concourse import bacc
from concourse.bass_utils import run_bass_kernel_spmd
from concourse.masks import make_identity

B, S, D, H, DK = 8, 1024, 1024, 16, 64
P = 128
F32 = mybir.dt.float32
BF16 = mybir.dt.bfloat16

INTERLEAVE = False
SKEW = 1

# Fallback flags (flip if a feature fails to compile/execute)
USE_XBAR = True        # dma_start_transpose for x^T (else PE transposes)
USE_DIVIDE = True      # tensor_scalar divide (else reciprocal + mul)
USE_CAST_DMA = True    # gpsimd casting DMA loads (else hwdge + convert)

# packed causal pt strip offsets: strip j holds q in [j*128, 1024)
PTOFF = [j * 1024 - 64 * j * (j - 1) for j in range(9)]  # PTOFF[8] = 4608


def _build_nc(mm_mode: str = "bf16"):
    nc = bacc.Bacc(
        "TRN2",
        target_bir_lowering=False,
        debug=False,
        enable_asserts=False,
        num_devices=B,
    )

    q_d = nc.dram_tensor("q", [S, D], F32, kind="ExternalInput")
    k_d = nc.dram_tensor("k", [S, D], F32, kind="ExternalInput")
    v_d = nc.dram_tensor("v", [S, D], F32, kind="ExternalInput")
    wq_d = nc.dram_tensor("w_q", [D, D], F32, kind="ExternalInput")
    wk_d = nc.dram_tensor("w_k", [D, D], F32, kind="ExternalInput")
    wv_d = nc.dram_tensor("w_v", [D, D], F32, kind="ExternalInput")
    wo_d = nc.dram_tensor("w_o", [D, D], F32, kind="ExternalInput")
    out_d = nc.dram_tensor("out", [S, D], F32, kind="ExternalOutput")

    # enlarge the SWDGE descriptor ring: 512-desc casting loads otherwise
    # fill the default 1024-desc carveout and serialize behind transfers
    nc.dynamic_dma_scratch_size = 16384 * 8

    x_ds = {"q": q_d, "k": k_d, "v": v_d}
    w_ds = {"q": wq_d, "k": wk_d, "v": wv_d}

    with tile.TileContext(nc) as tc:
        with (
            tc.tile_pool(name="consts", bufs=1) as consts,
            tc.tile_pool(name="wp", bufs=2) as wp,
            tc.tile_pool(name="xbfp", bufs=2) as xbfp,
            tc.tile_pool(name="xtp", bufs=2) as xtp,
            tc.tile_pool(name="projp", bufs=1) as projp,
            tc.tile_pool(name="proj2p", bufs=1) as proj2p,
            tc.tile_pool(name="optp", bufs=1) as optp,
            tc.tile_pool(name="t2p", bufs=2) as t2p,
            tc.tile_pool(name="ptp", bufs=2) as ptp,
            tc.tile_pool(name="vop", bufs=2) as vop,
            tc.tile_pool(name="hsp", bufs=2) as hsp,
            tc.tile_pool(name="ohtp", bufs=1) as ohtp,
            tc.tile_pool(name="outp", bufs=1) as outp,
            tc.tile_pool(name="psmm", bufs=3, space="PSUM") as psmm,
            tc.tile_pool(name="pspv", bufs=3, space="PSUM") as pspv,
            tc.tile_pool(name="psoh", bufs=2, space="PSUM") as psoh,
        ):
            ident = consts.tile([P, P], BF16, tag="ident")
            make_identity(nc, ident[:])

            # persistent vT2 [80, 1024]: rows 0:64 rewritten per head (V_h^T),
            # rows 64:80 stay 1.0 so the vo XBAR transpose lands the sumexp
            # ones column at position 64 of each 80-wide row.
            vT2 = consts.tile([80, 1024], BF16, tag="vT2")
            nc.gpsimd.memset(vT2[64:80, :], 1.0)

            # persistent projection outputs (transposed) + swapped twins
            pT = {t: projp.tile([P, 8, 1024], BF16, name=f"{t}pT", tag=f"{t}pT") for t in "qkv"}
            pT2 = {t: proj2p.tile([64, 8, 1024], BF16, name=f"{t}pT2", tag=f"{t}pT2") for t in "qkv"}
            opT = optp.tile([P, 8, 1024], BF16, tag="opT")

            drain_flip = [0]

            def drain(out_ap, in_ap, act_ok=False):
                # PSUM->SBUF drains (Pool has no PSUM access path).  ACT
                # copies are only used while ACT is otherwise idle (strip 0):
                # during phase 2 a copy whose matmul is late would
                # head-of-line-block the exp stream on ACT's in-order queue.
                drain_flip[0] += 1
                if act_ok and drain_flip[0] % 2 == 1:
                    nc.scalar.copy(out=out_ap, in_=in_ap)
                else:
                    nc.vector.tensor_copy(out=out_ap, in_=in_ap)

            def load_w(w_d, halves=2):
                # casting DMAs per weight (SWDGE desc-gen on Pool)
                w_sb = wp.tile([P, 8, 1024], BF16, tag="w")
                wsrc = w_d.ap().rearrange("(j p) c -> p j c", p=P)
                csz = 1024 // halves
                for chalf in range(halves):
                    c0 = chalf * csz
                    nc.gpsimd.dma_start(
                        w_sb[:, :, c0 : c0 + csz], wsrc[:, :, c0 : c0 + csz]
                    )
                return w_sb

            def strip_loads(t, st):
                """Load x strip as bf16 and XBAR-transpose into an xT tile."""
                s0 = st * 512
                x_d = x_ds[t]
                xT = xtp.tile([P, 8, 512], BF16, tag="xT")
                for half in range(2):
                    r0 = s0 + half * 256
                    xbf = xbfp.tile([P, 2, 1024], BF16, tag="xbf")
                    nc.gpsimd.dma_start(
                        xbf[:],
                        x_d.ap()[r0 : r0 + 256, :].rearrange(
                            "(s2 p) c -> p s2 c", p=P
                        ),
                    )
                    for s2 in range(2):
                        # one batched XBAR per x-tile: out[p, j, f] =
                        # x^T[j*128+p, f] (verified on HW).  All strip/store
                        # DMAs ride sync; per-head DMAs ride scalar, so a
                        # strip XBAR never queues behind head traffic (its
                        # completion gates the next xbf load via slot WAR).
                        s_lo = half * 256 + s2 * P
                        eng = nc.sync
                        eng.dma_start_transpose(
                            xT[:, :, s_lo : s_lo + P], xbf[:, s2, :]
                        )
                return xT

            def proj_unit(t, st, w_sb, xT, j):
                """One projection j-group (8 matmuls + drain); the last unit
                also emits the partition-swap twin DMA."""
                s0 = st * 512
                ps = psmm.tile([P, 512], F32, name="mm", tag="mm")
                for dit in range(8):
                    nc.tensor.matmul(
                        ps[:],
                        w_sb[:, dit, j * P : (j + 1) * P],
                        xT[:, dit, :],
                        start=(dit == 0),
                        stop=(dit == 7),
                    )
                drain(pT[t][:, j, s0 : s0 + 512], ps[:], act_ok=(st == 0))
                if j == 7:
                    # partition-swap twin: pT2[0:64] = pT[64:128]
                    eng = nc.sync
                    eng.dma_start(
                        pT2[t][0:64, :, s0 : s0 + 512],
                        pT[t][64:P, :, s0 : s0 + 512],
                    )

            def strip_proj(t, st, w_sb, xT):
                for j in range(8):
                    proj_unit(t, st, w_sb, xT, j)

            def head_front_a(h):
                """qT2/kT2/vT2 copies and the vo XBAR for head h."""
                t2 = {}
                for idx, t in enumerate("qkv"):
                    if t == "v":
                        dstT = vT2
                    else:
                        dstT = t2p.tile(
                            [64, 1024], BF16, name=f"{t}T2", tag=f"{t}T2"
                        )
                    t2[t] = dstT
                    dview = dstT[0:64, :].rearrange(
                        "u (a b1 b0) -> b0 u b1 a", a=64, b1=8, b0=2
                    )
                    for b0 in range(2):
                        src = (pT[t] if b0 == 0 else pT2[t])[
                            0:64, :, h * 64 : (h + 1) * 64
                        ]
                        eng = nc.gpsimd if (idx * 2 + b0) % 2 == 0 else nc.vector
                        eng.tensor_copy(out=dview[b0], in_=src)

                # vo[p, j, u] = V_h[j*128+p, u] for u<64, 1.0 at u=64 (sumexp)
                vo = vop.tile([P, 8, 80], BF16, name="vo", tag="vo")
                nc.scalar.dma_start_transpose(vo[:], vT2[:])
                pt = ptp.tile([P, PTOFF[8]], BF16, name="pt", tag="pt")
                return pt, vo, t2

            def score_chunks(h, pt, t2):
                """Emitter thunks for the 12 score-matmul + exp chunks."""
                chunks = []
                for j in range(8):
                    q0 = j * P
                    off = q0
                    while off < 1024:
                        n = min(512, 1024 - off)
                        def emit(j=j, q0=q0, off=off, n=n, first=(off == q0)):
                            ps = psmm.tile([P, 512], F32, name="mm", tag="mm")
                            nc.tensor.matmul(
                                ps[:, :n],
                                t2["k"][:, q0 : q0 + P],
                                t2["q"][:, off : off + n],
                                start=True,
                                stop=True,
                            )
                            nc.scalar.activation(
                                out=pt[:, PTOFF[j] + off - q0 : PTOFF[j] + off - q0 + n],
                                in_=ps[:, :n],
                                func=mybir.ActivationFunctionType.Exp,
                                scale=0.125,
                            )
                            if first:
                                nc.gpsimd.affine_select(
                                    out=pt[:, PTOFF[j] : PTOFF[j] + P],
                                    in_=pt[:, PTOFF[j] : PTOFF[j] + P],
                                    compare_op=mybir.AluOpType.is_ge,
                                    fill=0.0,
                                    base=0,
                                    pattern=[[1, P]],
                                    channel_multiplier=-1,
                                )
                        chunks.append(emit)
                        off += n
                return chunks

            def back_affines(h, pt, vo):
                pass

            def back_pv_quad(h, pt, vo, hs, quad):
                for iq in range(4):
                    i = quad * 4 + iq
                    pv = pspv.tile([P, DK + 1], F32, name="pv", tag="pv")
                    for j in range(i + 1):
                        nc.tensor.matmul(
                            pv[:],
                            pt[:, PTOFF[j] + (i - j) * P : PTOFF[j] + (i - j + 1) * P],
                            vo[:, j, : DK + 1],
                            start=(j == 0),
                            stop=(j == i),
                        )
                    # normalize straight out of PSUM on DVE (one hop
                    # shorter than staging through SBUF + Pool multiply)
                    rec = hsp.tile([P, 1], F32, name="rec", tag="rec")
                    nc.vector.reciprocal(rec[:], pv[:, DK : DK + 1])
                    nc.vector.tensor_scalar_mul(hs[:, i, :], pv[:, :DK], rec[:])

            def back_finish(h, hs):
                """Transpose to out_h^T, reorder-drain, scatter into opT."""
                psO = psoh.tile([64, 8, P], BF16, name="oht", tag="oht")
                for i in range(8):
                    nc.tensor.transpose(psO[:, i, :], hs[:, i, :], ident[:])
                # reorder drain: ohT[u, b*64 + i*8 + a'] = psO[u, i, a'*16 + b]
                ohT = ohtp.tile([64, 1024], BF16, name="ohT", tag="ohT")
                nc.vector.tensor_copy(
                    out=ohT[:].rearrange("u (b i a) -> u i a b", b=16, i=8, a=8),
                    in_=psO[:].rearrange("u i (a b) -> u i a b", a=8, b=16),
                )
                # two scatter DMAs into opT (by b parity)
                sview = ohT[:].rearrange("u (b1 b0 a) -> b0 u b1 a", b1=8, b0=2, a=64)
                for b0 in range(2):
                    eng = nc.scalar
                    eng.dma_start(
                        opT[b0 * 64 : (b0 + 1) * 64, :, h * 64 : (h + 1) * 64],
                        sview[b0],
                    )

            def p3_unit(blk, ch, wo_sb):
                ps = psmm.tile([P, 512], F32, name="mm", tag="mm")
                for pbt in range(8):
                    nc.tensor.matmul(
                        ps[:],
                        opT[:, pbt, blk * P : (blk + 1) * P],
                        wo_sb[:, pbt, ch * 512 : (ch + 1) * 512],
                        start=(pbt == 0),
                        stop=(pbt == 7),
                    )
                stg = outp.tile([P, 512], F32, name="ostg", tag="ostg")
                drain(stg[:], ps[:])
                eng = nc.sync
                eng.dma_start(
                    out_d.ap()[blk * P : (blk + 1) * P, ch * 512 : (ch + 1) * 512],
                    stg[:],
                )

            # ---------------- emission schedule ----------------
            # All six projection strips run up-front with loads interleaved
            # (each w loaded ONCE, reused by both strips: halves the weight
            # DMA), then all 16 heads run back-to-back with the output
            # projection as PE filler.  This un-bunches the DMA device,
            # which previously stalled the PE ~25us around the mid-phase
            # strip-1 reloads.
            xTq0 = strip_loads("q", 0)
            wq_sb = load_w(wq_d)
            xTq1 = strip_loads("q", 1)
            strip_proj("q", 0, wq_sb, xTq0)
            xTk0 = strip_loads("k", 0)
            strip_proj("q", 1, wq_sb, xTq1)
            wk_sb = load_w(wk_d)
            xTk1 = strip_loads("k", 1)
            strip_proj("k", 0, wk_sb, xTk0)
            wv_sb = load_w(wv_d)          # slot of wq: its readers are done
            xTv0 = strip_loads("v", 0)
            strip_proj("k", 1, wk_sb, xTk1)
            xTv1 = strip_loads("v", 1)
            strip_proj("v", 0, wv_sb, xTv0)
            wo_sb = load_w(wo_d)          # slot of wk: its readers are done

            # strip v1 is not needed until head 8: its 8 projection units
            # are woven into heads 0-2 (the heads phase is ACT-exp-bound,
            # so the PE slack there absorbs them), starting heads ~14us
            # earlier.
            v1u = [
                (lambda j=j: proj_unit("v", 1, wv_sb, xTv1, j))
                for j in range(8)
            ]

            def vpu():
                if v1u:
                    v1u.pop(0)()

            prev = None
            blocks_done = 0
            for h in range(H):
                pt, vo, t2 = head_front_a(h)
                sc = score_chunks(h, pt, t2)
                if prev is None:
                    sc[0](); sc[1](); sc[2](); vpu()
                    sc[3](); sc[4](); vpu()
                    sc[5](); sc[6](); vpu()
                    sc[7](); sc[8](); sc[9](); sc[10](); sc[11]()
                else:
                    h_b, pt_b, vo_b = prev
                    hs_b = hsp.tile([P, 8, DK], BF16, name="hs", tag="hs")
                    sc[0](); sc[1]()
                    back_pv_quad(h_b, pt_b, vo_b, hs_b, 0)
                    sc[2](); vpu(); sc[3](); sc[4](); sc[5]()
                    back_pv_quad(h_b, pt_b, vo_b, hs_b, 1)
                    sc[6](); vpu(); sc[7](); sc[8](); sc[9]()
                    back_finish(h_b, hs_b)
                    sc[10](); vpu(); sc[11]()
                if prev is not None:
                    if h >= 3:
                        b_ready = (h - 2) // 2
                        while blocks_done < min(b_ready, 7):
                            p3_unit(blocks_done, 0, wo_sb)
                            p3_unit(blocks_done, 1, wo_sb)
                            blocks_done += 1
                prev = (h, pt, vo)
            h_b, pt_b, vo_b = prev
            hs_b = hsp.tile([P, 8, DK], BF16, name="hs", tag="hs")
            back_pv_quad(h_b, pt_b, vo_b, hs_b, 0)
            back_pv_quad(h_b, pt_b, vo_b, hs_b, 1)
            back_finish(h_b, hs_b)
            while blocks_done < 8:
                p3_unit(blocks_done, 0, wo_sb)
                p3_unit(blocks_done, 1, wo_sb)
                blocks_done += 1

    if not nc.is_finalized():
        nc.finalize()
    return nc


_nc_cache = {}


def _get_nc(mm_mode="bf16"):
    if mm_mode not in _nc_cache:
        _nc_cache[mm_mode] = _build_nc(mm_mode)
    return _nc_cache[mm_mode]


MM_MODE = "bf16"


def kernel(q, k, v, mask, w_q, w_k, w_v, w_o, _trace=False):
    q = np.ascontiguousarray(np.asarray(q, dtype=np.float32))
    k = np.ascontiguousarray(np.asarray(k, dtype=np.float32))
    v = np.ascontiguousarray(np.asarray(v, dtype=np.float32))
    w_q = np.ascontiguousarray(np.asarray(w_q, dtype=np.float32))
    w_k = np.ascontiguousarray(np.asarray(w_k, dtype=np.float32))
    w_v = np.ascontiguousarray(np.asarray(w_v, dtype=np.float32))
    w_o = np.ascontiguousarray(np.asarray(w_o, dtype=np.float32))

    nc = _get_nc()
    in_maps = [
        {
            "q": q[i],
            "k": k[i],
            "v": v[i],
            "w_q": w_q,
            "w_k": w_k,
            "w_v": w_v,
            "w_o": w_o,
        }
        for i in range(B)
    ]
    res = run_bass_kernel_spmd(
        nc, in_maps, core_ids=list(range(B)), trace=_trace
    )
    out = np.stack([r["out"] for r in res.results], axis=0)
    if _trace:
        kernel.last_exec_time_ns = res.exec_time_ns
        kernel.last_trace = res.instructions_and_trace
    return out
